# revision 10
# baseline (speedup 1.0000x reference)
"""Causal multi-head attention on 8 Trainium2 NeuronCores.

Tensor-parallel over heads: 16 heads -> 2 heads per core (128 of the 1024
model dims per core). Each core computes q/k/v projections for its head
slice, causal attention, and its partial output projection (row-slice of
Wo); the host sums the 8 partials (+bo, supplied to core 0 only as data).

Per-core layouts (partition dim first):
  xT     [1024, 8192]   x transposed (host-prepared), contraction on partitions
  qT/kT  [128, 2048]/b  per-batch, head dims on partitions (h0: 0-63, h1: 64-127)
  vplus  [128, 16, 130] per-batch: [v_h0(64) | ones | v_h1(64) | ones] per key tile
  scores sT = k @ qT    [128 keys, 512 q] x 2 heads, row-packed concurrent MMs
  ctxT   [65, 512] psum: rows 0-63 = unnormalized ctx^T, row 64 = softmax denom
All matmuls run as float32r (TF32-like, full PE rate at N>=512).
"""

import numpy as np
from contextlib import ExitStack

import concourse.bass as bass
import concourse.mybir as mybir
import concourse.tile as tile
from concourse import bacc
from concourse import bass_utils
from concourse.masks import make_identity

F32R = mybir.dt.float32r
F32 = mybir.dt.float32
AF = mybir.ActivationFunctionType

B, S, D = 4, 2048, 1024
H, DH = 16, 64
NCORES = 8
DHC = 128           # head dims per core (2 heads x 64)
BS = B * S          # 8192
QC = 512            # q-chunk width
NQC = S // QC       # 4 q-chunks per batch
NKT = S // 128      # 16 key tiles per batch
NKD = D // 128      # 8 contraction tiles for projections
MASKVAL = -1.0e12

_CACHE = {}


def _build():
    nc = bacc.Bacc("TRN2", target_bir_lowering=False, debug=False)
    xT = nc.dram_tensor("xT", [D, BS], F32R, kind="ExternalInput").ap()
    wqkv = nc.dram_tensor("wqkv", [D, 3 * DHC], F32R, kind="ExternalInput").ap()
    bqkv = nc.dram_tensor("bqkv", [DHC, 3], F32, kind="ExternalInput").ap()
    wo = nc.dram_tensor("wo", [DHC, D], F32R, kind="ExternalInput").ap()
    cmask = nc.dram_tensor("cmask", [128, 4, 2 * QC], F32R, kind="ExternalInput").ap()
    out = nc.dram_tensor("out", [BS, D], F32, kind="ExternalOutput").ap()

    with tile.TileContext(nc) as tc:
        with ExitStack() as ctx:
            consts = ctx.enter_context(tc.tile_pool(name="consts", bufs=1))
            big = ctx.enter_context(tc.tile_pool(name="big", bufs=2))
            bigc = ctx.enter_context(tc.tile_pool(name="bigc", bufs=3))
            work = ctx.enter_context(tc.tile_pool(name="work", bufs=16))
            expp = ctx.enter_context(tc.tile_pool(name="expp", bufs=6))
            outp = ctx.enter_context(tc.tile_pool(name="outp", bufs=3))
            small = ctx.enter_context(tc.tile_pool(name="small", bufs=4))
            psum = ctx.enter_context(tc.tile_pool(name="psum", bufs=1, space="PSUM"))
            psum2 = ctx.enter_context(tc.tile_pool(name="psum2", bufs=2, space="PSUM"))

            # ---- constants ----
            t_w = consts.tile([128, NKD, 3 * DHC], F32R, tag="w")
            nc.gpsimd.dma_start(t_w, wqkv.rearrange("(t p) c -> p t c", p=128))
            t_bqkv = consts.tile([DHC, 3], F32, tag="bqkv")
            nc.gpsimd.dma_start(t_bqkv, bqkv)
            t_wo = consts.tile([DHC, D], F32R, tag="wo")
            nc.gpsimd.dma_start(t_wo, wo)
            t_mask = consts.tile([128, 4, 2 * QC], F32R, tag="mask")
            nc.gpsimd.dma_start(t_mask, cmask)
            t_idf = consts.tile([128, 128], F32, tag="idf")
            make_identity(nc, t_idf)
            t_id = consts.tile([128, 128], F32R, tag="id")
            nc.vector.tensor_copy(t_id, t_idf)
            t_ones = consts.tile([128, 1], F32, tag="ones")
            nc.vector.memset(t_ones, 1.0)

            qT = {}
            kT = {}
            vplus = {}
            ctxT = {}

            def emit_proj_qc(b, qc):
                """QKV projections for batch b, seq chunk qc (512 wide)."""
                if qc == 0:
                    qT[b] = big.tile([128, S], F32R, tag="qT", name=f"qT{b}")
                    kT[b] = big.tile([128, S], F32R, tag="kT", name=f"kT{b}")
                    vplus[b] = big.tile([128, NKT, 130], F32R, tag="vplus",
                                        name=f"vplus{b}")
                    nc.vector.tensor_copy(
                        vplus[b][:, :, 64:65],
                        t_ones[:, None, :].broadcast_to([128, NKT, 1]),
                    )
                    nc.vector.tensor_copy(
                        vplus[b][:, :, 129:130],
                        t_ones[:, None, :].broadcast_to([128, NKT, 1]),
                    )
                j0 = b * S + qc * QC
                xts = []
                for kd in range(NKD):
                    xt = work.tile([128, QC], F32R, tag="xt")
                    nc.sync.dma_start(xt, xT[kd * 128:(kd + 1) * 128, j0:j0 + QC])
                    xts.append(xt)
                for pi in range(3):
                    ps = psum2.tile([128, QC], F32, tag="pp", bufs=1)
                    for kd in range(NKD):
                        nc.tensor.matmul(
                            ps,
                            t_w[:, kd, pi * DHC:(pi + 1) * DHC],
                            xts[kd],
                            start=(kd == 0),
                            stop=(kd == NKD - 1),
                        )
                    bias_ap = t_bqkv[:, pi:pi + 1]
                    if pi == 0:
                        nc.vector.tensor_scalar_add(
                            qT[b][:, qc * QC:(qc + 1) * QC], ps, bias_ap
                        )
                    elif pi == 1:
                        nc.vector.tensor_scalar_add(
                            kT[b][:, qc * QC:(qc + 1) * QC], ps, bias_ap
                        )
                    else:
                        vst = small.tile([128, QC], F32R, tag="vstage")
                        nc.vector.tensor_scalar_add(vst, ps, bias_ap)
                        for tt in range(4):
                            loc = qc * 4 + tt  # key-tile index within batch
                            ps_t = psum2.tile([128, 128], F32R, tag="pp", name="ps_t", bufs=1)
                            nc.tensor.transpose(
                                ps_t, vst[:, tt * 128:(tt + 1) * 128], t_id
                            )
                            nc.vector.tensor_copy(
                                vplus[b][:, loc, 0:64], ps_t[:, 0:64]
                            )
                            nc.vector.tensor_copy(
                                vplus[b][:, loc, 65:129], ps_t[:, 64:128]
                            )

            def emit_attn_qc(b, qc):
                """Causal attention + out-proj for batch b, q-chunk qc."""
                if qc == 0:
                    ctxT[b] = bigc.tile([128, S], F32R, tag="ctxT", name=f"ctxT{b}")
                q0 = qc * QC
                ps_c0 = psum.tile([65, QC], F32, tag="ctx", name="ps_c0", bufs=3)
                ps_c1 = psum.tile([65, QC], F32, tag="ctx", name="ps_c1", bufs=3)
                nkt = 4 * qc + 4
                for kt in range(nkt):
                    ps_s = psum2.tile([128, 2 * QC], F32, tag="scores")
                    nc.tensor.matmul(
                        ps_s[:, 0:QC],
                        kT[b][0:64, kt * 128:(kt + 1) * 128],
                        qT[b][0:64, q0:q0 + QC],
                        start=True, stop=True,
                    )
                    nc.tensor.matmul(
                        ps_s[:, QC:2 * QC],
                        kT[b][64:128, kt * 128:(kt + 1) * 128],
                        qT[b][64:128, q0:q0 + QC],
                        start=True, stop=True,
                        tile_position=(64, 0),
                    )
                    t_exp = expp.tile([128, 2 * QC], F32R, tag="exp")
                    nc.scalar.activation(t_exp, ps_s, AF.Exp, scale=0.125)
                    o = kt - 4 * qc
                    if o >= 0:  # diagonal tile: zero the causal-masked region
                        nc.vector.tensor_mul(t_exp, t_exp, t_mask[:, o, :])
                    nc.tensor.matmul(
                        ps_c0, vplus[b][:, kt, 0:65], t_exp[:, 0:QC],
                        start=(kt == 0), stop=(kt == nkt - 1),
                    )
                    nc.tensor.matmul(
                        ps_c1, vplus[b][:, kt, 65:130], t_exp[:, QC:2 * QC],
                        start=(kt == 0), stop=(kt == nkt - 1),
                    )
                # softmax normalization (denominator in row 64)
                for h, ps_c in ((0, ps_c0), (1, ps_c1)):
                    t_d = small.tile([1, QC], F32, tag="den")
                    nc.vector.tensor_copy(t_d, ps_c[64:65, :])
                    t_r = small.tile([1, QC], F32, tag="recip")
                    nc.vector.reciprocal_approx_fast(t_r, t_d)
                    t_bc = small.tile([64, QC], F32, tag="bcast")
                    nc.gpsimd.partition_broadcast(t_bc, t_r)
                    nc.vector.tensor_mul(
                        ctxT[b][h * 64:(h + 1) * 64, q0:q0 + QC],
                        ps_c[0:64, :],
                        t_bc,
                    )
                # out-projection for this qc's 4 q-tiles
                for qi in range(QC // 128):
                    qt = qc * 4 + qi
                    r0 = b * S + qt * 128
                    for ch in range(2):
                        ps_o = psum2.tile([128, QC], F32, tag="pp", name="ps_o", bufs=1)
                        nc.tensor.matmul(
                            ps_o,
                            ctxT[b][:, qt * 128:(qt + 1) * 128],
                            t_wo[:, ch * QC:(ch + 1) * QC],
                            start=True, stop=True,
                        )
                        t_o = outp.tile([128, QC], F32, tag="out")
                        if (qt * 2 + ch) % 2 == 0:
                            nc.vector.tensor_copy(t_o, ps_o)
                        else:
                            nc.scalar.copy(t_o, ps_o)
                        nc.sync.dma_start(out[r0:r0 + 128, ch * QC:(ch + 1) * QC], t_o)

            def emit_proj(b):
                for qc in range(NQC):
                    emit_proj_qc(b, qc)

            def emit_attn(b):
                for qc in range(NQC):
                    emit_attn_qc(b, qc)

            emit_proj(0)
            emit_proj(1)
            emit_attn(0)
            emit_proj(2)
            emit_attn(1)
            emit_proj(3)
            emit_attn(2)
            emit_attn(3)

    nc.compile()
    return nc


def _host_inputs(x, wq, bq, wk, bk, wv, bv, wo, bo):
    x = np.asarray(x, dtype=np.float32).reshape(BS, D)
    xT = np.ascontiguousarray(x.T)
    # causal masks for the 4 diagonal offsets, duplicated for the 2 heads
    p = np.arange(128)[:, None]
    j = np.arange(QC)[None, :]
    cmask = np.zeros((128, 4, 2 * QC), dtype=np.float32)
    for o in range(4):
        m = (j >= p + o * 128).astype(np.float32)
        cmask[:, o, 0:QC] = m
        cmask[:, o, QC:2 * QC] = m
    wq, wk, wv, wo = (np.asarray(a, dtype=np.float32) for a in (wq, wk, wv, wo))
    bq, bk, bv, bo = (np.asarray(a, dtype=np.float32) for a in (bq, bk, bv, bo))
    in_maps = []
    for c in range(NCORES):
        sl = slice(c * DHC, (c + 1) * DHC)
        wqkv = np.ascontiguousarray(
            np.concatenate([wq[:, sl], wk[:, sl], wv[:, sl]], axis=1)
        )
        bqkv = np.ascontiguousarray(np.stack([bq[sl], bk[sl], bv[sl]], axis=1))
        in_maps.append({
            "xT": xT,
            "wqkv": wqkv,
            "bqkv": bqkv,
            "wo": np.ascontiguousarray(wo[sl, :]),
            "cmask": cmask,
        })
    return in_maps


def kernel(x, wq, bq, wk, bk, wv, bv, wo, bo, _trace=False, _tmpdir=None):
    if "nc" not in _CACHE:
        _CACHE["nc"] = _build()
    nc = _CACHE["nc"]
    in_maps = _host_inputs(x, wq, bq, wk, bk, wv, bv, wo, bo)
    res = bass_utils.run_bass_kernel_spmd(
        nc, in_maps, core_ids=list(range(NCORES)), trace=_trace, tmpdir=_tmpdir
    )
    _CACHE["last_results"] = res
    acc = np.zeros((BS, D), dtype=np.float64)
    for c in range(NCORES):
        acc += res.results[c]["out"]
    acc += np.asarray(bo, dtype=np.float64)[None, :]
    return acc.astype(np.float32).reshape(B, S, D)


# revision 11
# speedup vs baseline: 1.2494x; 1.2494x over previous
"""Causal multi-head attention on 8 Trainium2 NeuronCores.

Tensor-parallel over heads: 16 heads -> 2 heads per core (128 of the 1024
model dims per core). Each core computes q/k/v projections for its head
slice, causal attention, and its partial output projection (row-slice of
Wo); the host sums the 8 partials (+bo, supplied to core 0 only as data).

Per-core layouts (partition dim first):
  xT     [1024, 8192]   x transposed (host-prepared), contraction on partitions
  qT/kT  [128, 2048]/b  per-batch, head dims on partitions (h0: 0-63, h1: 64-127)
  vplus  [128, 16, 130] per-batch: [v_h0(64) | ones | v_h1(64) | ones] per key tile
  scores sT = k @ qT    [128 keys, 512 q] x 2 heads, row-packed concurrent MMs
  ctxT   [65, 512] psum: rows 0-63 = unnormalized ctx^T, row 64 = softmax denom
All matmuls run as float32r (TF32-like, full PE rate at N>=512).
"""

import numpy as np
from contextlib import ExitStack

import concourse.bass as bass
import concourse.mybir as mybir
import concourse.tile as tile
from concourse import bacc
from concourse import bass_utils
from concourse.masks import make_identity

F32R = mybir.dt.float32r
F32 = mybir.dt.float32
AF = mybir.ActivationFunctionType

B, S, D = 4, 2048, 1024
H, DH = 16, 64
NCORES = 8
DHC = 128           # head dims per core (2 heads x 64)
BS = B * S          # 8192
QC = 512            # q-chunk width
NQC = S // QC       # 4 q-chunks per batch
NKT = S // 128      # 16 key tiles per batch
NKD = D // 128      # 8 contraction tiles for projections
MASKVAL = -1.0e12

_CACHE = {}


def _build():
    nc = bacc.Bacc("TRN2", target_bir_lowering=False, debug=False)
    xT = nc.dram_tensor("xT", [D, BS], F32R, kind="ExternalInput").ap()
    wqkv = nc.dram_tensor("wqkv", [D, 3 * DHC], F32R, kind="ExternalInput").ap()
    bqkv = nc.dram_tensor("bqkv", [DHC, 3], F32, kind="ExternalInput").ap()
    wo = nc.dram_tensor("wo", [DHC, D], F32R, kind="ExternalInput").ap()
    cmask = nc.dram_tensor("cmask", [128, 4, 2 * QC], F32R, kind="ExternalInput").ap()
    out = nc.dram_tensor("out", [BS, D], F32, kind="ExternalOutput").ap()

    with tile.TileContext(nc) as tc:
        with ExitStack() as ctx:
            consts = ctx.enter_context(tc.tile_pool(name="consts", bufs=1))
            big = ctx.enter_context(tc.tile_pool(name="big", bufs=2))
            bigc = ctx.enter_context(tc.tile_pool(name="bigc", bufs=3))
            work = ctx.enter_context(tc.tile_pool(name="work", bufs=16))
            expp = ctx.enter_context(tc.tile_pool(name="expp", bufs=6))
            outp = ctx.enter_context(tc.tile_pool(name="outp", bufs=3))
            small = ctx.enter_context(tc.tile_pool(name="small", bufs=4))
            psum = ctx.enter_context(tc.tile_pool(name="psum", bufs=1, space="PSUM"))
            psum2 = ctx.enter_context(tc.tile_pool(name="psum2", bufs=2, space="PSUM"))

            # ---- constants ----
            t_w = consts.tile([128, NKD, 3 * DHC], F32R, tag="w")
            nc.gpsimd.dma_start(t_w, wqkv.rearrange("(t p) c -> p t c", p=128))
            t_bqkv = consts.tile([DHC, 3], F32, tag="bqkv")
            nc.gpsimd.dma_start(t_bqkv, bqkv)
            t_wo = consts.tile([DHC, D], F32R, tag="wo")
            nc.gpsimd.dma_start(t_wo, wo)
            t_mask = consts.tile([128, 4, 2 * QC], F32R, tag="mask")
            nc.gpsimd.dma_start(t_mask, cmask)
            t_idf = consts.tile([128, 128], F32, tag="idf")
            make_identity(nc, t_idf)
            t_id = consts.tile([128, 128], F32R, tag="id")
            nc.vector.tensor_copy(t_id, t_idf)
            t_ones = consts.tile([128, 1], F32, tag="ones")
            nc.vector.memset(t_ones, 1.0)

            qT = {}
            kT = {}
            vplus = {}
            ctxT = {}

            def emit_proj_qc(b, qc):
                """QKV projections for batch b, seq chunk qc (512 wide)."""
                if qc == 0:
                    qT[b] = big.tile([128, S], F32R, tag="qT", name=f"qT{b}")
                    kT[b] = big.tile([128, S], F32R, tag="kT", name=f"kT{b}")
                    vplus[b] = big.tile([128, NKT, 130], F32R, tag="vplus",
                                        name=f"vplus{b}")
                    nc.vector.tensor_copy(
                        vplus[b][:, :, 64:65],
                        t_ones[:, None, :].broadcast_to([128, NKT, 1]),
                    )
                    nc.vector.tensor_copy(
                        vplus[b][:, :, 129:130],
                        t_ones[:, None, :].broadcast_to([128, NKT, 1]),
                    )
                j0 = b * S + qc * QC
                xts = []
                for kd in range(NKD):
                    xt = work.tile([128, QC], F32R, tag="xt")
                    nc.sync.dma_start(xt, xT[kd * 128:(kd + 1) * 128, j0:j0 + QC])
                    xts.append(xt)
                for pi in range(3):
                    ps = psum2.tile([128, QC], F32, tag="pp")
                    for kd in range(NKD):
                        nc.tensor.matmul(
                            ps,
                            t_w[:, kd, pi * DHC:(pi + 1) * DHC],
                            xts[kd],
                            start=(kd == 0),
                            stop=(kd == NKD - 1),
                        )
                    bias_ap = t_bqkv[:, pi:pi + 1]
                    if pi == 0:
                        nc.vector.tensor_scalar_add(
                            qT[b][:, qc * QC:(qc + 1) * QC], ps, bias_ap
                        )
                    elif pi == 1:
                        nc.vector.tensor_scalar_add(
                            kT[b][:, qc * QC:(qc + 1) * QC], ps, bias_ap
                        )
                    else:
                        vst = small.tile([128, QC], F32R, tag="vstage")
                        nc.vector.tensor_scalar_add(vst, ps, bias_ap)
                        for tt in range(4):
                            loc = qc * 4 + tt  # key-tile index within batch
                            ps_t = psum2.tile([128, 128], F32R, tag="pp", name="ps_t")
                            nc.tensor.transpose(
                                ps_t, vst[:, tt * 128:(tt + 1) * 128], t_id
                            )
                            nc.vector.tensor_copy(
                                vplus[b][:, loc, 0:64], ps_t[:, 0:64]
                            )
                            nc.vector.tensor_copy(
                                vplus[b][:, loc, 65:129], ps_t[:, 64:128]
                            )

            def emit_attn_qc(b, qc):
                """Causal attention + out-proj for batch b, q-chunk qc."""
                if qc == 0:
                    ctxT[b] = bigc.tile([128, S], F32R, tag="ctxT", name=f"ctxT{b}")
                q0 = qc * QC
                ps_c0 = psum.tile([65, QC], F32, tag="ctx0")
                ps_c1 = psum.tile([65, QC], F32, tag="ctx1")
                nkt = 4 * qc + 4
                for kt in range(nkt):
                    ps_s = psum2.tile([128, 2 * QC], F32, tag="scores")
                    nc.tensor.matmul(
                        ps_s[:, 0:QC],
                        kT[b][0:64, kt * 128:(kt + 1) * 128],
                        qT[b][0:64, q0:q0 + QC],
                        start=True, stop=True,
                    )
                    nc.tensor.matmul(
                        ps_s[:, QC:2 * QC],
                        kT[b][64:128, kt * 128:(kt + 1) * 128],
                        qT[b][64:128, q0:q0 + QC],
                        start=True, stop=True,
                        tile_position=(64, 0),
                    )
                    t_exp = expp.tile([128, 2 * QC], F32R, tag="exp")
                    nc.scalar.activation(t_exp, ps_s, AF.Exp, scale=0.125)
                    o = kt - 4 * qc
                    if o >= 0:  # diagonal tile: zero the causal-masked region
                        nc.vector.tensor_mul(t_exp, t_exp, t_mask[:, o, :])
                    nc.tensor.matmul(
                        ps_c0, vplus[b][:, kt, 0:65], t_exp[:, 0:QC],
                        start=(kt == 0), stop=(kt == nkt - 1),
                    )
                    nc.tensor.matmul(
                        ps_c1, vplus[b][:, kt, 65:130], t_exp[:, QC:2 * QC],
                        start=(kt == 0), stop=(kt == nkt - 1),
                    )
                # softmax normalization (denominator in row 64)
                for h, ps_c in ((0, ps_c0), (1, ps_c1)):
                    t_d = small.tile([1, QC], F32, tag="den")
                    nc.vector.tensor_copy(t_d, ps_c[64:65, :])
                    t_r = small.tile([1, QC], F32, tag="recip")
                    nc.vector.reciprocal_approx_fast(t_r, t_d)
                    t_bc = small.tile([64, QC], F32, tag="bcast")
                    nc.gpsimd.partition_broadcast(t_bc, t_r)
                    nc.vector.tensor_mul(
                        ctxT[b][h * 64:(h + 1) * 64, q0:q0 + QC],
                        ps_c[0:64, :],
                        t_bc,
                    )
                # out-projection for this qc's 4 q-tiles
                for qi in range(QC // 128):
                    qt = qc * 4 + qi
                    r0 = b * S + qt * 128
                    for ch in range(2):
                        ps_o = psum2.tile([128, QC], F32, tag="pp", name="ps_o")
                        nc.tensor.matmul(
                            ps_o,
                            ctxT[b][:, qt * 128:(qt + 1) * 128],
                            t_wo[:, ch * QC:(ch + 1) * QC],
                            start=True, stop=True,
                        )
                        t_o = outp.tile([128, QC], F32, tag="out")
                        if (qt * 2 + ch) % 2 == 0:
                            nc.vector.tensor_copy(t_o, ps_o)
                        else:
                            nc.scalar.copy(t_o, ps_o)
                        nc.sync.dma_start(out[r0:r0 + 128, ch * QC:(ch + 1) * QC], t_o)

            def emit_proj(b):
                for qc in range(NQC):
                    emit_proj_qc(b, qc)

            def emit_attn(b):
                for qc in range(NQC):
                    emit_attn_qc(b, qc)

            emit_proj(0)
            emit_proj(1)
            emit_attn(0)
            emit_proj(2)
            emit_attn(1)
            emit_proj(3)
            emit_attn(2)
            emit_attn(3)

    nc.compile()
    return nc


def _host_inputs(x, wq, bq, wk, bk, wv, bv, wo, bo):
    x = np.asarray(x, dtype=np.float32).reshape(BS, D)
    xT = np.ascontiguousarray(x.T)
    # causal masks for the 4 diagonal offsets, duplicated for the 2 heads
    p = np.arange(128)[:, None]
    j = np.arange(QC)[None, :]
    cmask = np.zeros((128, 4, 2 * QC), dtype=np.float32)
    for o in range(4):
        m = (j >= p + o * 128).astype(np.float32)
        cmask[:, o, 0:QC] = m
        cmask[:, o, QC:2 * QC] = m
    wq, wk, wv, wo = (np.asarray(a, dtype=np.float32) for a in (wq, wk, wv, wo))
    bq, bk, bv, bo = (np.asarray(a, dtype=np.float32) for a in (bq, bk, bv, bo))
    in_maps = []
    for c in range(NCORES):
        sl = slice(c * DHC, (c + 1) * DHC)
        wqkv = np.ascontiguousarray(
            np.concatenate([wq[:, sl], wk[:, sl], wv[:, sl]], axis=1)
        )
        bqkv = np.ascontiguousarray(np.stack([bq[sl], bk[sl], bv[sl]], axis=1))
        in_maps.append({
            "xT": xT,
            "wqkv": wqkv,
            "bqkv": bqkv,
            "wo": np.ascontiguousarray(wo[sl, :]),
            "cmask": cmask,
        })
    return in_maps


def kernel(x, wq, bq, wk, bk, wv, bv, wo, bo, _trace=False, _tmpdir=None):
    if "nc" not in _CACHE:
        _CACHE["nc"] = _build()
    nc = _CACHE["nc"]
    in_maps = _host_inputs(x, wq, bq, wk, bk, wv, bv, wo, bo)
    res = bass_utils.run_bass_kernel_spmd(
        nc, in_maps, core_ids=list(range(NCORES)), trace=_trace, tmpdir=_tmpdir
    )
    _CACHE["last_results"] = res
    acc = np.zeros((BS, D), dtype=np.float64)
    for c in range(NCORES):
        acc += res.results[c]["out"]
    acc += np.asarray(bo, dtype=np.float64)[None, :]
    return acc.astype(np.float32).reshape(B, S, D)
